# revision 15
# baseline (speedup 1.0000x reference)
"""Trainium2 Bass kernel for an 8-expert top-2 MoE layer with shared expert.

Strategy (expert-parallel, per the sharding hint):
  - Host computes the (tiny) router: logits = x @ gate_w.T, softmax, top-2,
    normalized combine weights, and the auxiliary losses (0.03% of FLOPs).
  - Tokens are dispatched (gathered) by top-k expert id: core e receives the
    tokens routed to expert e (padded to capacity C, a multiple of 128),
    plus expert e's FFN weights.
  - The shared expert is data-parallel: core c processes token block c
    (T/8 tokens) with replicated shared weights (down-proj pre-scaled by
    sigmoid(shared_gate_scalar) on host).
  - Each core runs gate/up matmuls + SiLU*up + down matmul in fp16 with fp32
    PSUM accumulation; routed outputs are scaled on-device by the combine
    weight. Host un-permutes with two row-gathers and adds the shared output.

Shapes (hardcoded per the problem spec):
  hidden_states [4, 2048, 1024], E=8 experts, I=2048, H=1024, top-k=2.
"""

import numpy as np
from contextlib import ExitStack

B, S, H, I, E, TOPK = 4, 2048, 1024, 2048, 8, 2
T = B * S            # 8192 tokens
SH = T // 8          # shared-expert tokens per core
P = 128
HT = H // P          # 8 partition tiles over H
IT = I // P          # 16 partition tiles over I
EPS = 1e-9
NCORES = 8

_BUILD_CACHE: dict = {}   # C -> (nc, exec_fn or None)


# ---------------------------------------------------------------------------
# Device kernel
# ---------------------------------------------------------------------------

def _build_nc(C: int, SH: int = SH, use_silu: bool = True, repeats: int = 1):
    import concourse.bass as bass  # noqa: F401
    import concourse.tile as tile
    from concourse import bacc, mybir

    MD = mybir.dt.float16
    F32 = mybir.dt.float32
    AF = mybir.ActivationFunctionType

    nc = bacc.Bacc("TRN2", target_bir_lowering=False, debug=False,
                   num_devices=NCORES)

    xe = nc.dram_tensor("xe", [H, C], MD, kind="ExternalInput")
    xs = nc.dram_tensor("xs", [H, SH], MD, kind="ExternalInput")
    sc = nc.dram_tensor("sc", [C], F32, kind="ExternalInput")
    wg = nc.dram_tensor("wg", [H, I], MD, kind="ExternalInput")
    wu = nc.dram_tensor("wu", [H, I], MD, kind="ExternalInput")
    wd = nc.dram_tensor("wd", [I, H], MD, kind="ExternalInput")
    sg = nc.dram_tensor("sg", [H, I], MD, kind="ExternalInput")
    su = nc.dram_tensor("su", [H, I], MD, kind="ExternalInput")
    sd = nc.dram_tensor("sd", [I, H], MD, kind="ExternalInput")
    ye = nc.dram_tensor("ye", [C, H], F32, kind="ExternalOutput")
    ys = nc.dram_tensor("ys", [SH, H], F32, kind="ExternalOutput")

    with tile.TileContext(nc) as tc, ExitStack() as ctx:
        wpool = ctx.enter_context(tc.tile_pool(name="weights", bufs=1))
        xpool = ctx.enter_context(tc.tile_pool(name="x", bufs=3))
        hpool = ctx.enter_context(tc.tile_pool(name="h", bufs=2))
        tpool = ctx.enter_context(tc.tile_pool(name="tmp", bufs=4))
        opool = ctx.enter_context(tc.tile_pool(name="yout", bufs=4))
        pgpool = ctx.enter_context(tc.tile_pool(name="pg", bufs=2, space="PSUM"))
        pupool = ctx.enter_context(tc.tile_pool(name="pu", bufs=2, space="PSUM"))
        pdpool = ctx.enter_context(tc.tile_pool(name="pd", bufs=3, space="PSUM"))

        def load_weight(dram, rows, cols, kind):
            # routed and shared weights share slots (same tags): the shared
            # set is loaded into the same SBUF space once phase 1 releases it
            tiles = []
            for r in range(rows // P):
                t = wpool.tile([P, cols], MD, tag=f"w_{kind}_{r}")
                nc.sync.dma_start(t[:], dram.ap()[r * P:(r + 1) * P, :])
                tiles.append(t)
            return tiles

        sc_box = [None]

        def ffn(x_dram, n_tok, g_tiles, u_tiles, d_tiles, scaled, out_dram):
            sc_tile = sc_box[0]
            n_chunks = (n_tok + 511) // 512
            for ci in range(n_chunks):
                w = min(512, n_tok - ci * 512)
                # load x chunk: 8 tiles [128, w] of x^T
                xt = []
                for ht in range(HT):
                    t = xpool.tile([P, 512], MD, tag=f"x_{ht}")
                    nc.sync.dma_start(
                        t[:, :w],
                        x_dram.ap()[ht * P:(ht + 1) * P, ci * 512:ci * 512 + w])
                    xt.append(t)
                # gate/up projections + silu*up -> h^T (fp16), tiles over I
                hts = []
                for it in range(IT):
                    pg_ = pgpool.tile([P, 512], F32, tag="pg")
                    pu_ = pupool.tile([P, 512], F32, tag="pu")
                    for ht in range(HT):
                        nc.tensor.matmul(
                            pg_[:, :w], g_tiles[ht][:, it * P:(it + 1) * P],
                            xt[ht][:, :w], start=(ht == 0), stop=(ht == HT - 1))
                    for ht in range(HT):
                        nc.tensor.matmul(
                            pu_[:, :w], u_tiles[ht][:, it * P:(it + 1) * P],
                            xt[ht][:, :w], start=(ht == 0), stop=(ht == HT - 1))
                    st = tpool.tile([P, 512], F32, tag="silu")
                    if use_silu:
                        nc.scalar.activation(st[:, :w], pg_[:, :w], AF.Silu)
                    else:
                        # CoreSim has no Silu: sigmoid then an extra multiply
                        nc.scalar.activation(st[:, :w], pg_[:, :w], AF.Sigmoid)
                        nc.vector.tensor_mul(st[:, :w], st[:, :w], pg_[:, :w])
                    h_ = hpool.tile([P, 512], MD, tag=f"h_{it}")
                    nc.vector.tensor_mul(h_[:, :w], st[:, :w], pu_[:, :w])
                    hts.append(h_)
                # down projection: tokens back on partitions
                for mt in range(w // P):
                    for hh in range(2):
                        pd_ = pdpool.tile([P, 512], F32, tag="pd")
                        for it in range(IT):
                            nc.tensor.matmul(
                                pd_[:], hts[it][:, mt * P:(mt + 1) * P],
                                d_tiles[it][:, hh * 512:(hh + 1) * 512],
                                start=(it == 0), stop=(it == IT - 1))
                        ot = opool.tile([P, 512], F32, tag="yo")
                        if scaled:
                            nc.vector.tensor_scalar_mul(
                                ot[:], pd_[:],
                                sc_tile[:, ci * 4 + mt:ci * 4 + mt + 1])
                        else:
                            nc.scalar.copy(ot[:], pd_[:])
                        row0 = ci * 512 + mt * P
                        nc.sync.dma_start(
                            out_dram.ap()[row0:row0 + P,
                                          hh * 512:(hh + 1) * 512], ot[:])

        def body():
            # combine-weight scales: one [128, C//128] tile, column j covers
            # tokens j*128 .. j*128+127
            sc_tile = wpool.tile([P, C // P], F32, tag="scales")
            nc.sync.dma_start(sc_tile[:],
                              sc.ap().rearrange("(m p) -> p m", p=P))
            sc_box[0] = sc_tile

            wg_t = load_weight(wg, H, I, "g")
            wu_t = load_weight(wu, H, I, "u")
            wd_t = load_weight(wd, I, H, "d")
            ffn(xe, C, wg_t, wu_t, wd_t, True, ye)

            sg_t = load_weight(sg, H, I, "g")
            su_t = load_weight(su, H, I, "u")
            sd_t = load_weight(sd, I, H, "d")
            ffn(xs, SH, sg_t, su_t, sd_t, False, ys)

        if repeats == 1:
            body()
        else:
            # hardware loop for benchmarking: body emitted once, run R times
            with tc.For_i(0, repeats, 1):
                body()

    nc.compile()
    return nc


# ---------------------------------------------------------------------------
# Execution (PJRT via axon); cached jitted callable with library fallback
# ---------------------------------------------------------------------------

IN_NAMES = ["xe", "xs", "sc", "wg", "wu", "wd", "sg", "su", "sd"]
OUT_NAMES = ["ye", "ys"]


def _make_exec(nc):
    """Build a reusable jitted 8-core executor for `nc`.

    Modeled on concourse.bass2jax.run_bass_via_pjrt (multi-core branch), but
    built once so repeated calls don't re-trace/re-compile.
    """
    import jax
    import numpy as _np
    from jax.sharding import Mesh, PartitionSpec
    from jax.experimental.shard_map import shard_map
    from concourse import bass2jax, mybir

    bass2jax.install_neuronx_cc_hook()

    in_names = list(IN_NAMES) + list(OUT_NAMES)
    out_avals = []
    zero_shapes = []
    for alloc in nc.m.functions[0].allocations:
        if not isinstance(alloc, mybir.MemoryLocationSet):
            continue
        if alloc.kind == "ExternalOutput":
            name = alloc.memorylocations[0].name
            assert name in OUT_NAMES
            shape = tuple(alloc.tensor_shape)
            dtype = mybir.dt.np(alloc.dtype)
            out_avals.append((name, jax.core.ShapedArray(shape, dtype)))
    # keep OUT_NAMES order
    out_avals = {n: a for n, a in out_avals}
    avals = tuple(out_avals[n] for n in OUT_NAMES)
    zero_shapes = [(a.shape, a.dtype) for a in avals]

    def _body(*args):
        outs = bass2jax._bass_exec_p.bind(
            *args,
            out_avals=avals,
            in_names=tuple(in_names),
            out_names=tuple(OUT_NAMES),
            lowering_input_output_aliases=(),
            sim_require_finite=True,
            sim_require_nnan=True,
            nc=nc,
        )
        return tuple(outs)

    devices = jax.devices()[:NCORES]
    mesh = Mesh(_np.asarray(devices), ("core",))
    n_args = len(in_names)
    sharded = jax.jit(
        shard_map(_body, mesh=mesh,
                  in_specs=(PartitionSpec("core"),) * n_args,
                  out_specs=(PartitionSpec("core"),) * len(OUT_NAMES),
                  check_rep=False),
        keep_unused=True,
    )

    def pack(in_maps):
        args = []
        for name in IN_NAMES:
            args.append(_np.concatenate([in_maps[c][name] for c in
                                         range(NCORES)], axis=0))
        for shape, dtype in zero_shapes:
            args.append(_np.zeros((NCORES * shape[0],) + shape[1:], dtype))
        return args

    def unpack(out_arrs):
        res = []
        for c in range(NCORES):
            d = {}
            for i, name in enumerate(OUT_NAMES):
                shape = zero_shapes[i][0]
                d[name] = _np.asarray(out_arrs[i]).reshape(
                    (NCORES,) + shape)[c]
            res.append(d)
        return res

    return sharded, pack, unpack


def _execute(nc, in_maps):
    """Run the SPMD kernel on cores 0-7; returns list of per-core dicts."""
    import os
    key = id(nc)
    if not os.environ.get("KERNEL_FASTPATH"):
        from concourse.bass_utils import run_bass_kernel_spmd
        res = run_bass_kernel_spmd(nc, in_maps, core_ids=list(range(NCORES)))
        return res.results
    try:
        cached = _BUILD_CACHE.get(("exec", key))
        if cached is None:
            cached = _make_exec(nc)
            _BUILD_CACHE[("exec", key)] = cached
        sharded, pack, unpack = cached
        out = sharded(*pack(in_maps))
        import jax
        jax.block_until_ready(out)
        return unpack(out)
    except Exception:
        from concourse.bass_utils import run_bass_kernel_spmd
        res = run_bass_kernel_spmd(nc, in_maps, core_ids=list(range(NCORES)))
        return res.results


# ---------------------------------------------------------------------------
# Host-side routing / dispatch / combine
# ---------------------------------------------------------------------------

def _route(x, gate_w):
    """Router + aux losses in fp32 numpy, mirroring the reference ops."""
    logits = x @ gate_w.T                                    # [T, E]
    m = logits.max(-1, keepdims=True)
    ex = np.exp(logits - m)
    probs = ex / ex.sum(-1, keepdims=True)                   # softmax
    # top-2 (ties: lower index first, like jax.lax.top_k)
    idx = np.argsort(-probs, axis=-1, kind="stable")[:, :TOPK]
    pv = np.take_along_axis(probs, idx, axis=-1)
    wts = pv / (pv.sum(-1, keepdims=True) + np.float32(EPS))

    # aux losses (all fp32)
    counts = np.bincount(idx.ravel(), minlength=E).astype(np.float32)
    tokens_per_expert = counts / np.float32(T * TOPK + EPS)
    avg_probs = probs.mean(0)
    load_balance = np.float32(E) * (tokens_per_expert * avg_probs).sum()
    lse = (m[:, 0] + np.log(ex.sum(-1)))
    z_loss = np.mean(lse ** 2) * np.float32(0.001)
    entropy = -(probs * np.log(probs + np.float32(EPS))).sum(-1).mean()
    entropy_loss = (np.log(np.float32(E)) - entropy) * np.float32(0.01)
    usage = (tokens_per_expert > 0.01).astype(np.float32).mean()
    util_loss = (np.float32(1.0) - usage) * np.float32(0.1)
    aux = np.float32(load_balance + z_loss + entropy_loss + util_loss)
    return idx, wts, aux


def _prepare(hidden_states, gate_w, expert_gate, expert_up, expert_down,
             shared_gate_w, shared_up_w, shared_down_w, shared_gate_scalar):
    """Host routing + dispatch. Returns (in_maps, meta) where meta carries
    what _combine needs."""
    x = np.ascontiguousarray(np.asarray(hidden_states, np.float32)
                             .reshape(T, H))
    gate_w = np.asarray(gate_w, np.float32)

    idx, wts, aux = _route(x, gate_w)

    # ---- dispatch: group (token, k) pairs by expert ----
    flat_e = idx.ravel()
    flat_w = wts.ravel().astype(np.float32)
    flat_t = np.repeat(np.arange(T, dtype=np.int64), TOPK)
    order = np.argsort(flat_e, kind="stable")
    counts = np.bincount(flat_e, minlength=E)
    starts = np.zeros(E + 1, np.int64)
    np.cumsum(counts, out=starts[1:])
    C = int(max(P, -(-counts.max() // P) * P))

    slot = np.empty(TOPK * T, np.int64)
    for e in range(E):
        slot[order[starts[e]:starts[e + 1]]] = np.arange(counts[e])

    x16 = x.astype(np.float16)
    sig = 1.0 / (1.0 + np.exp(-np.float32(shared_gate_scalar[0])))

    def w16T(a):  # [r, c] fp32 -> [c, r] fp16 contiguous
        return np.ascontiguousarray(np.asarray(a, np.float32).T
                                    .astype(np.float16))

    sg16 = w16T(shared_gate_w)
    su16 = w16T(shared_up_w)
    sd16 = np.ascontiguousarray(
        (np.asarray(shared_down_w, np.float32) * sig).T.astype(np.float16))

    in_maps = []
    for e in range(E):
        sel = order[starts[e]:starts[e + 1]]
        tok = flat_t[sel]
        xpad = np.zeros((C, H), np.float16)
        xpad[:counts[e]] = x16[tok]
        scv = np.zeros(C, np.float32)
        scv[:counts[e]] = flat_w[sel]
        in_maps.append({
            "xe": np.ascontiguousarray(xpad.T),
            "xs": np.ascontiguousarray(x16[e * SH:(e + 1) * SH].T),
            "sc": scv,
            "wg": w16T(expert_gate[e]),
            "wu": w16T(expert_up[e]),
            "wd": w16T(expert_down[e]),
            "sg": sg16, "su": su16, "sd": sd16,
        })

    contrib = (flat_e.astype(np.int64) * C + slot).reshape(T, TOPK)
    meta = {"C": C, "contrib": contrib, "aux": aux}
    return in_maps, meta


def _combine(results, meta):
    """Un-permute routed outputs (two row-gathers) + add shared output."""
    C, contrib, aux = meta["C"], meta["contrib"], meta["aux"]
    ye_all = np.concatenate([results[e]["ye"] for e in range(E)], axis=0)
    final = ye_all[contrib[:, 0]]
    final += ye_all[contrib[:, 1]]
    final += np.concatenate([results[c]["ys"] for c in range(NCORES)], axis=0)
    final = final.reshape(B, S, H).astype(np.float32)
    return final, np.float32(aux)


def kernel(hidden_states, gate_w, expert_gate, expert_up, expert_down,
           shared_gate_w, shared_up_w, shared_down_w, shared_gate_scalar):
    in_maps, meta = _prepare(hidden_states, gate_w, expert_gate, expert_up,
                             expert_down, shared_gate_w, shared_up_w,
                             shared_down_w, shared_gate_scalar)
    C = meta["C"]
    nc = _BUILD_CACHE.get(C)
    if nc is None:
        nc = _build_nc(C)
        _BUILD_CACHE[C] = nc
    results = _execute(nc, in_maps)
    return _combine(results, meta)


# revision 18
# speedup vs baseline: 1.5004x; 1.5004x over previous
"""Trainium2 Bass kernel for an 8-expert top-2 MoE layer with shared expert.

Strategy (expert-parallel, per the sharding hint):
  - Host computes the (tiny) router: logits = x @ gate_w.T, softmax, top-2,
    normalized combine weights, and the auxiliary losses (0.03% of FLOPs).
  - Tokens are dispatched (gathered) by top-k expert id: core e receives the
    tokens routed to expert e (padded to capacity C, a multiple of 128),
    plus expert e's FFN weights.
  - The shared expert is data-parallel: core c processes token block c
    (T/8 tokens) with replicated shared weights (down-proj pre-scaled by
    sigmoid(shared_gate_scalar) on host).
  - Each core runs gate/up matmuls + SiLU*up + down matmul in fp16 with fp32
    PSUM accumulation; routed outputs are scaled on-device by the combine
    weight. Host un-permutes with two row-gathers and adds the shared output.

All DRAM tensors use partition-major tiled layouts prepared on the host so
that every DMA is a single fully-contiguous transfer (the per-descriptor /
per-queue issue overhead otherwise dominates: measured 167 GB/s/core for
512 KB strided loads vs ~340 GB/s for big linear ones).

Shapes (hardcoded per the problem spec):
  hidden_states [4, 2048, 1024], E=8 experts, I=2048, H=1024, top-k=2.
"""

import numpy as np
from contextlib import ExitStack

B, S, H, I, E, TOPK = 4, 2048, 1024, 2048, 8, 2
T = B * S            # 8192 tokens
SH = T // 8          # shared-expert tokens per core
P = 128
HT = H // P          # 8 partition tiles over H
IT = I // P          # 16 partition tiles over I
EPS = 1e-9
NCORES = 8

_BUILD_CACHE: dict = {}   # C -> nc


def _chunks(n_tok):
    out = []
    o = 0
    while o < n_tok:
        out.append((o, min(512, n_tok - o)))
        o += 512
    return out


# ---------------------------------------------------------------------------
# Host-side packing helpers (layouts shared by device builder and host prep)
# ---------------------------------------------------------------------------

def _pack_w(mat_T):
    """[R, Cc] (fp16) -> [128, R//128, Cc] partition-major tiled, contiguous."""
    R, Cc = mat_T.shape
    return np.ascontiguousarray(
        mat_T.reshape(R // P, P, Cc).transpose(1, 0, 2))


def _pack_x(xT, n_tok):
    """xT [H, n_tok] fp16 -> flat blocks [ci][ht][128][w], one contiguous
    run per (ci, ht) DMA."""
    parts = []
    for o, w in _chunks(n_tok):
        blk = xT[:, o:o + w].reshape(HT, P, w)
        parts.append(blk.reshape(-1))
    return np.concatenate(parts)


def _unpack_y(flat, n_tok):
    """flat [tt][hh][128][512] fp32 -> [n_tok, H]."""
    return flat.reshape(n_tok // P, 2, P, 512).transpose(0, 2, 1, 3) \
               .reshape(n_tok, H)


# ---------------------------------------------------------------------------
# Device kernel
# ---------------------------------------------------------------------------

def _build_nc(C: int, SH: int = SH, use_silu: bool = True, repeats: int = 1,
              do_routed: bool = True, do_shared: bool = True):
    import concourse.bass as bass  # noqa: F401
    import concourse.tile as tile
    from concourse import bacc, mybir

    MD = mybir.dt.float16
    F32 = mybir.dt.float32
    AF = mybir.ActivationFunctionType

    nc = bacc.Bacc("TRN2", target_bir_lowering=False, debug=False,
                   num_devices=NCORES)

    xe = nc.dram_tensor("xe", [H * C], MD, kind="ExternalInput")
    xs = nc.dram_tensor("xs", [H * SH], MD, kind="ExternalInput")
    sc = nc.dram_tensor("sc", [C], F32, kind="ExternalInput")
    wg = nc.dram_tensor("wg", [P, HT, I], MD, kind="ExternalInput")
    wu = nc.dram_tensor("wu", [P, HT, I], MD, kind="ExternalInput")
    wd = nc.dram_tensor("wd", [P, IT, H], MD, kind="ExternalInput")
    sg = nc.dram_tensor("sg", [P, HT, I], MD, kind="ExternalInput")
    su = nc.dram_tensor("su", [P, HT, I], MD, kind="ExternalInput")
    sd = nc.dram_tensor("sd", [P, IT, H], MD, kind="ExternalInput")
    ye = nc.dram_tensor("ye", [C * H], F32, kind="ExternalOutput")
    ys = nc.dram_tensor("ys", [SH * H], F32, kind="ExternalOutput")

    with tile.TileContext(nc) as tc, ExitStack() as ctx:
        wpool = ctx.enter_context(tc.tile_pool(name="weights", bufs=1))
        xpool = ctx.enter_context(tc.tile_pool(name="x", bufs=3))
        hpool = ctx.enter_context(tc.tile_pool(name="h", bufs=2))
        tpool = ctx.enter_context(tc.tile_pool(name="tmp", bufs=4))
        opool = ctx.enter_context(tc.tile_pool(name="yout", bufs=4))
        pgpool = ctx.enter_context(tc.tile_pool(name="pg", bufs=2, space="PSUM"))
        pupool = ctx.enter_context(tc.tile_pool(name="pu", bufs=2, space="PSUM"))
        pdpool = ctx.enter_context(tc.tile_pool(name="pd", bufs=3, space="PSUM"))

        sc_box = [None]

        def load_weights(g_dram, u_dram, d_dram):
            # one contiguous DMA per matrix, spread across the three
            # DMA-issuing queues (gpsimd SWDGE, ACT HWDGE, SP HWDGE)
            gt = wpool.tile([P, HT, I], MD, tag="w_g")
            nc.gpsimd.dma_start(gt[:], g_dram.ap())
            ut = wpool.tile([P, HT, I], MD, tag="w_u")
            nc.scalar.dma_start(ut[:], u_dram.ap())
            dt_ = wpool.tile([P, IT, H], MD, tag="w_d")
            nc.gpsimd.dma_start(dt_[:], d_dram.ap())
            return gt, ut, dt_

        def ffn(x_dram, n_tok, gt, ut, dt_, scaled, out_dram):
            sc_tile = sc_box[0]
            for ci, (off, w) in enumerate(_chunks(n_tok)):
                base = off * H
                # x chunk: 8 contiguous DMAs of [128, w]
                xt = []
                for ht in range(HT):
                    t = xpool.tile([P, 512], MD, tag=f"x_{ht}")
                    src = x_dram.ap()[base + ht * P * w:
                                      base + (ht + 1) * P * w]
                    nc.sync.dma_start(t[:, :w],
                                      src.rearrange("(p c) -> p c", p=P))
                    xt.append(t)
                # gate/up projections + silu*up -> h^T (fp16), tiles over I
                hts = []
                for it in range(IT):
                    pg_ = pgpool.tile([P, 512], F32, tag="pg")
                    pu_ = pupool.tile([P, 512], F32, tag="pu")
                    for ht in range(HT):
                        nc.tensor.matmul(
                            pg_[:, :w], gt[:, ht, it * P:(it + 1) * P],
                            xt[ht][:, :w], start=(ht == 0), stop=(ht == HT - 1))
                    for ht in range(HT):
                        nc.tensor.matmul(
                            pu_[:, :w], ut[:, ht, it * P:(it + 1) * P],
                            xt[ht][:, :w], start=(ht == 0), stop=(ht == HT - 1))
                    st = tpool.tile([P, 512], F32, tag="silu")
                    if use_silu:
                        nc.scalar.activation(st[:, :w], pg_[:, :w], AF.Silu)
                    else:
                        # CoreSim has no Silu: sigmoid then an extra multiply
                        nc.scalar.activation(st[:, :w], pg_[:, :w], AF.Sigmoid)
                        nc.vector.tensor_mul(st[:, :w], st[:, :w], pg_[:, :w])
                    h_ = hpool.tile([P, 512], MD, tag=f"h_{it}")
                    nc.vector.tensor_mul(h_[:, :w], st[:, :w], pu_[:, :w])
                    hts.append(h_)
                # down projection: tokens back on partitions
                for mt in range(w // P):
                    tt = off // P + mt
                    for hh in range(2):
                        pd_ = pdpool.tile([P, 512], F32, tag="pd")
                        for it in range(IT):
                            nc.tensor.matmul(
                                pd_[:], hts[it][:, mt * P:(mt + 1) * P],
                                dt_[:, it, hh * 512:(hh + 1) * 512],
                                start=(it == 0), stop=(it == IT - 1))
                        ot = opool.tile([P, 512], F32, tag="yo")
                        if scaled:
                            nc.vector.tensor_scalar_mul(
                                ot[:], pd_[:], sc_tile[:, tt:tt + 1])
                        else:
                            nc.vector.tensor_copy(ot[:], pd_[:])
                        dst = out_dram.ap()[(tt * 2 + hh) * P * 512:
                                            (tt * 2 + hh + 1) * P * 512]
                        nc.scalar.dma_start(
                            dst.rearrange("(p c) -> p c", p=P), ot[:])

        def body():
            # combine-weight scales: one [128, C//128] tile, column j covers
            # tokens j*128 .. j*128+127
            sc_tile = wpool.tile([P, C // P], F32, tag="scales")
            nc.sync.dma_start(sc_tile[:],
                              sc.ap().rearrange("(m p) -> p m", p=P))
            sc_box[0] = sc_tile
            if do_routed:
                gt, ut, dt_ = load_weights(wg, wu, wd)
                ffn(xe, C, gt, ut, dt_, True, ye)
            if do_shared:
                gt, ut, dt_ = load_weights(sg, su, sd)
                ffn(xs, SH, gt, ut, dt_, False, ys)

        if repeats == 1:
            body()
        else:
            # hardware loop for benchmarking: body emitted once, run R times
            with tc.For_i(0, repeats, 1):
                body()

    nc.compile()
    return nc


# ---------------------------------------------------------------------------
# Execution (PJRT via axon through the concourse library path)
# ---------------------------------------------------------------------------

IN_NAMES = ["xe", "xs", "sc", "wg", "wu", "wd", "sg", "su", "sd"]
OUT_NAMES = ["ye", "ys"]


def _execute(nc, in_maps):
    """Run the SPMD kernel on cores 0-7; returns list of per-core dicts."""
    from concourse.bass_utils import run_bass_kernel_spmd
    res = run_bass_kernel_spmd(nc, in_maps, core_ids=list(range(NCORES)))
    return res.results


# ---------------------------------------------------------------------------
# Host-side routing / dispatch / combine
# ---------------------------------------------------------------------------

def _route(x, gate_w):
    """Router + aux losses in fp32 numpy, mirroring the reference ops."""
    logits = x @ gate_w.T                                    # [T, E]
    m = logits.max(-1, keepdims=True)
    ex = np.exp(logits - m)
    probs = ex / ex.sum(-1, keepdims=True)                   # softmax
    # top-2 (ties: lower index first, like jax.lax.top_k)
    idx = np.argsort(-probs, axis=-1, kind="stable")[:, :TOPK]
    pv = np.take_along_axis(probs, idx, axis=-1)
    wts = pv / (pv.sum(-1, keepdims=True) + np.float32(EPS))

    # aux losses (all fp32)
    counts = np.bincount(idx.ravel(), minlength=E).astype(np.float32)
    tokens_per_expert = counts / np.float32(T * TOPK + EPS)
    avg_probs = probs.mean(0)
    load_balance = np.float32(E) * (tokens_per_expert * avg_probs).sum()
    lse = (m[:, 0] + np.log(ex.sum(-1)))
    z_loss = np.mean(lse ** 2) * np.float32(0.001)
    entropy = -(probs * np.log(probs + np.float32(EPS))).sum(-1).mean()
    entropy_loss = (np.log(np.float32(E)) - entropy) * np.float32(0.01)
    usage = (tokens_per_expert > 0.01).astype(np.float32).mean()
    util_loss = (np.float32(1.0) - usage) * np.float32(0.1)
    aux = np.float32(load_balance + z_loss + entropy_loss + util_loss)
    return idx, wts, aux


def _prepare(hidden_states, gate_w, expert_gate, expert_up, expert_down,
             shared_gate_w, shared_up_w, shared_down_w, shared_gate_scalar):
    """Host routing + dispatch. Returns (in_maps, meta) where meta carries
    what _combine needs."""
    x = np.ascontiguousarray(np.asarray(hidden_states, np.float32)
                             .reshape(T, H))
    gate_w = np.asarray(gate_w, np.float32)

    idx, wts, aux = _route(x, gate_w)

    # ---- dispatch: group (token, k) pairs by expert ----
    flat_e = idx.ravel()
    flat_w = wts.ravel().astype(np.float32)
    flat_t = np.repeat(np.arange(T, dtype=np.int64), TOPK)
    order = np.argsort(flat_e, kind="stable")
    counts = np.bincount(flat_e, minlength=E)
    starts = np.zeros(E + 1, np.int64)
    np.cumsum(counts, out=starts[1:])
    C = int(max(P, -(-counts.max() // P) * P))

    slot = np.empty(TOPK * T, np.int64)
    for e in range(E):
        slot[order[starts[e]:starts[e + 1]]] = np.arange(counts[e])

    x16 = x.astype(np.float16)
    sig = 1.0 / (1.0 + np.exp(-np.float32(shared_gate_scalar[0])))

    def w16T(a):  # [r, c] fp32 -> [c, r] fp16 contiguous
        return np.asarray(a, np.float32).T.astype(np.float16)

    sg16 = _pack_w(w16T(shared_gate_w))
    su16 = _pack_w(w16T(shared_up_w))
    sd16 = _pack_w((np.asarray(shared_down_w, np.float32) * sig).T
                   .astype(np.float16))

    in_maps = []
    for e in range(E):
        sel = order[starts[e]:starts[e + 1]]
        tok = flat_t[sel]
        xpad = np.zeros((C, H), np.float16)
        xpad[:counts[e]] = x16[tok]
        scv = np.zeros(C, np.float32)
        scv[:counts[e]] = flat_w[sel]
        in_maps.append({
            "xe": _pack_x(np.ascontiguousarray(xpad.T), C),
            "xs": _pack_x(np.ascontiguousarray(x16[e * SH:(e + 1) * SH].T),
                          SH),
            "sc": scv,
            "wg": _pack_w(w16T(expert_gate[e])),
            "wu": _pack_w(w16T(expert_up[e])),
            "wd": _pack_w(w16T(expert_down[e])),
            "sg": sg16, "su": su16, "sd": sd16,
        })

    contrib = (flat_e.astype(np.int64) * C + slot).reshape(T, TOPK)
    meta = {"C": C, "contrib": contrib, "aux": aux}
    return in_maps, meta


def _combine(results, meta):
    """Un-permute routed outputs (two row-gathers) + add shared output."""
    C, contrib, aux = meta["C"], meta["contrib"], meta["aux"]
    ye_all = np.concatenate([_unpack_y(results[e]["ye"], C)
                             for e in range(E)], axis=0)
    final = ye_all[contrib[:, 0]]
    final += ye_all[contrib[:, 1]]
    final += np.concatenate([_unpack_y(results[c]["ys"], SH)
                             for c in range(NCORES)], axis=0)
    final = final.reshape(B, S, H).astype(np.float32)
    return final, np.float32(aux)


def kernel(hidden_states, gate_w, expert_gate, expert_up, expert_down,
           shared_gate_w, shared_up_w, shared_down_w, shared_gate_scalar):
    in_maps, meta = _prepare(hidden_states, gate_w, expert_gate, expert_up,
                             expert_down, shared_gate_w, shared_up_w,
                             shared_down_w, shared_gate_scalar)
    C = meta["C"]
    nc = _BUILD_CACHE.get(C)
    if nc is None:
        nc = _build_nc(C)
        _BUILD_CACHE[C] = nc
    results = _execute(nc, in_maps)
    return _combine(results, meta)


# revision 19
# speedup vs baseline: 1.5241x; 1.0158x over previous
"""Trainium2 Bass kernel for an 8-expert top-2 MoE layer with shared expert.

Strategy (expert-parallel, per the sharding hint):
  - Host computes the (tiny) router: logits = x @ gate_w.T, softmax, top-2,
    normalized combine weights, and the auxiliary losses (0.03% of FLOPs).
  - Tokens are dispatched (gathered) by top-k expert id: core e receives the
    tokens routed to expert e (padded to capacity C, a multiple of 128),
    plus expert e's FFN weights.
  - The shared expert is data-parallel: core c processes token block c
    (T/8 tokens) with replicated shared weights (down-proj pre-scaled by
    sigmoid(shared_gate_scalar) on host).
  - Each core runs gate/up matmuls + SiLU*up + down matmul in fp16 with fp32
    PSUM accumulation; routed outputs are scaled on-device by the combine
    weight. Host un-permutes with two row-gathers and adds the shared output.

All DRAM tensors use partition-major tiled layouts prepared on the host so
that every DMA is a single fully-contiguous transfer (the per-descriptor /
per-queue issue overhead otherwise dominates: measured 167 GB/s/core for
512 KB strided loads vs ~340 GB/s for big linear ones).

Shapes (hardcoded per the problem spec):
  hidden_states [4, 2048, 1024], E=8 experts, I=2048, H=1024, top-k=2.
"""

import numpy as np
from contextlib import ExitStack

B, S, H, I, E, TOPK = 4, 2048, 1024, 2048, 8, 2
T = B * S            # 8192 tokens
SH = T // 8          # shared-expert tokens per core
P = 128
HT = H // P          # 8 partition tiles over H
IT = I // P          # 16 partition tiles over I
EPS = 1e-9
NCORES = 8

_BUILD_CACHE: dict = {}   # C -> nc


def _chunks(n_tok):
    out = []
    o = 0
    while o < n_tok:
        out.append((o, min(512, n_tok - o)))
        o += 512
    return out


# ---------------------------------------------------------------------------
# Host-side packing helpers (layouts shared by device builder and host prep)
# ---------------------------------------------------------------------------

def _pack_w(mat_T):
    """[R, Cc] (fp16) -> [128, R//128, Cc] partition-major tiled, contiguous."""
    R, Cc = mat_T.shape
    return np.ascontiguousarray(
        mat_T.reshape(R // P, P, Cc).transpose(1, 0, 2))


def _pack_x(xT, n_tok):
    """xT [H, n_tok] fp16 -> flat blocks [ci][ht][128][w], one contiguous
    run per (ci, ht) DMA."""
    parts = []
    for o, w in _chunks(n_tok):
        blk = xT[:, o:o + w].reshape(HT, P, w)
        parts.append(blk.reshape(-1))
    return np.concatenate(parts)


def _unpack_y(flat, n_tok):
    """flat [tt][hh][128][512] fp16 -> [n_tok, H]."""
    return flat.reshape(n_tok // P, 2, P, 512).transpose(0, 2, 1, 3) \
               .reshape(n_tok, H)


# ---------------------------------------------------------------------------
# Device kernel
# ---------------------------------------------------------------------------

def _build_nc(C: int, SH: int = SH, use_silu: bool = True, repeats: int = 1,
              do_routed: bool = True, do_shared: bool = True):
    import concourse.bass as bass  # noqa: F401
    import concourse.tile as tile
    from concourse import bacc, mybir

    MD = mybir.dt.float16
    F32 = mybir.dt.float32
    AF = mybir.ActivationFunctionType

    nc = bacc.Bacc("TRN2", target_bir_lowering=False, debug=False,
                   num_devices=NCORES)

    xe = nc.dram_tensor("xe", [H * C], MD, kind="ExternalInput")
    xs = nc.dram_tensor("xs", [H * SH], MD, kind="ExternalInput")
    sc = nc.dram_tensor("sc", [C], F32, kind="ExternalInput")
    wg = nc.dram_tensor("wg", [P, HT, I], MD, kind="ExternalInput")
    wu = nc.dram_tensor("wu", [P, HT, I], MD, kind="ExternalInput")
    wd = nc.dram_tensor("wd", [P, IT, H], MD, kind="ExternalInput")
    sg = nc.dram_tensor("sg", [P, HT, I], MD, kind="ExternalInput")
    su = nc.dram_tensor("su", [P, HT, I], MD, kind="ExternalInput")
    sd = nc.dram_tensor("sd", [P, IT, H], MD, kind="ExternalInput")
    ye = nc.dram_tensor("ye", [C * H], MD, kind="ExternalOutput")
    ys = nc.dram_tensor("ys", [SH * H], MD, kind="ExternalOutput")

    with tile.TileContext(nc) as tc, ExitStack() as ctx:
        wpool = ctx.enter_context(tc.tile_pool(name="weights", bufs=1))
        xpool = ctx.enter_context(tc.tile_pool(name="x", bufs=3))
        hpool = ctx.enter_context(tc.tile_pool(name="h", bufs=2))
        tpool = ctx.enter_context(tc.tile_pool(name="tmp", bufs=4))
        opool = ctx.enter_context(tc.tile_pool(name="yout", bufs=4))
        pgpool = ctx.enter_context(tc.tile_pool(name="pg", bufs=2, space="PSUM"))
        pupool = ctx.enter_context(tc.tile_pool(name="pu", bufs=2, space="PSUM"))
        pdpool = ctx.enter_context(tc.tile_pool(name="pd", bufs=4, space="PSUM"))

        sc_box = [None]

        def load_weights(g_dram, u_dram, d_dram):
            # one contiguous DMA per matrix, spread across the three
            # DMA-issuing queues (gpsimd SWDGE, ACT HWDGE, SP HWDGE)
            gt = wpool.tile([P, HT, I], MD, tag="w_g")
            nc.gpsimd.dma_start(gt[:], g_dram.ap())
            ut = wpool.tile([P, HT, I], MD, tag="w_u")
            nc.scalar.dma_start(ut[:], u_dram.ap())
            dt_ = wpool.tile([P, IT, H], MD, tag="w_d")
            nc.gpsimd.dma_start(dt_[:], d_dram.ap())
            return gt, ut, dt_

        def ffn(x_dram, n_tok, gt, ut, dt_, scaled, out_dram):
            sc_tile = sc_box[0]
            for ci, (off, w) in enumerate(_chunks(n_tok)):
                base = off * H
                # x chunk: 8 contiguous DMAs of [128, w]
                xt = []
                for ht in range(HT):
                    t = xpool.tile([P, 512], MD, tag=f"x_{ht}")
                    src = x_dram.ap()[base + ht * P * w:
                                      base + (ht + 1) * P * w]
                    nc.sync.dma_start(t[:, :w],
                                      src.rearrange("(p c) -> p c", p=P))
                    xt.append(t)
                # gate/up projections + silu*up -> h^T (fp16), tiles over I
                hts = []
                for it in range(IT):
                    pg_ = pgpool.tile([P, 512], F32, tag="pg")
                    pu_ = pupool.tile([P, 512], F32, tag="pu")
                    for ht in range(HT):
                        nc.tensor.matmul(
                            pg_[:, :w], gt[:, ht, it * P:(it + 1) * P],
                            xt[ht][:, :w], start=(ht == 0), stop=(ht == HT - 1))
                    for ht in range(HT):
                        nc.tensor.matmul(
                            pu_[:, :w], ut[:, ht, it * P:(it + 1) * P],
                            xt[ht][:, :w], start=(ht == 0), stop=(ht == HT - 1))
                    st = tpool.tile([P, 512], F32, tag="silu")
                    if use_silu:
                        nc.scalar.activation(st[:, :w], pg_[:, :w], AF.Silu)
                    else:
                        # CoreSim has no Silu: sigmoid then an extra multiply
                        nc.scalar.activation(st[:, :w], pg_[:, :w], AF.Sigmoid)
                        nc.vector.tensor_mul(st[:, :w], st[:, :w], pg_[:, :w])
                    h_ = hpool.tile([P, 512], MD, tag=f"h_{it}")
                    nc.vector.tensor_mul(h_[:, :w], st[:, :w], pu_[:, :w])
                    hts.append(h_)
                # down projection: tokens back on partitions
                for mt in range(w // P):
                    tt = off // P + mt
                    for hh in range(2):
                        pd_ = pdpool.tile([P, 512], F32, tag="pd")
                        for it in range(IT):
                            nc.tensor.matmul(
                                pd_[:], hts[it][:, mt * P:(mt + 1) * P],
                                dt_[:, it, hh * 512:(hh + 1) * 512],
                                start=(it == 0), stop=(it == IT - 1))
                        ot = opool.tile([P, 512], MD, tag="yo")
                        if scaled:
                            nc.vector.tensor_scalar_mul(
                                ot[:], pd_[:], sc_tile[:, tt:tt + 1])
                        else:
                            nc.vector.tensor_copy(ot[:], pd_[:])
                        dst = out_dram.ap()[(tt * 2 + hh) * P * 512:
                                            (tt * 2 + hh + 1) * P * 512]
                        nc.scalar.dma_start(
                            dst.rearrange("(p c) -> p c", p=P), ot[:])

        def body():
            # combine-weight scales: one [128, C//128] tile, column j covers
            # tokens j*128 .. j*128+127
            sc_tile = wpool.tile([P, C // P], F32, tag="scales")
            nc.sync.dma_start(sc_tile[:],
                              sc.ap().rearrange("(m p) -> p m", p=P))
            sc_box[0] = sc_tile
            if do_routed:
                gt, ut, dt_ = load_weights(wg, wu, wd)
                ffn(xe, C, gt, ut, dt_, True, ye)
            if do_shared:
                gt, ut, dt_ = load_weights(sg, su, sd)
                ffn(xs, SH, gt, ut, dt_, False, ys)

        if repeats == 1:
            body()
        else:
            # hardware loop for benchmarking: body emitted once, run R times
            with tc.For_i(0, repeats, 1):
                body()

    nc.compile()
    return nc


# ---------------------------------------------------------------------------
# Execution (PJRT via axon through the concourse library path)
# ---------------------------------------------------------------------------

IN_NAMES = ["xe", "xs", "sc", "wg", "wu", "wd", "sg", "su", "sd"]
OUT_NAMES = ["ye", "ys"]


def _execute(nc, in_maps):
    """Run the SPMD kernel on cores 0-7; returns list of per-core dicts."""
    from concourse.bass_utils import run_bass_kernel_spmd
    res = run_bass_kernel_spmd(nc, in_maps, core_ids=list(range(NCORES)))
    return res.results


# ---------------------------------------------------------------------------
# Host-side routing / dispatch / combine
# ---------------------------------------------------------------------------

def _route(x, gate_w):
    """Router + aux losses in fp32 numpy, mirroring the reference ops."""
    logits = x @ gate_w.T                                    # [T, E]
    m = logits.max(-1, keepdims=True)
    ex = np.exp(logits - m)
    probs = ex / ex.sum(-1, keepdims=True)                   # softmax
    # top-2 (ties: lower index first, like jax.lax.top_k)
    idx = np.argsort(-probs, axis=-1, kind="stable")[:, :TOPK]
    pv = np.take_along_axis(probs, idx, axis=-1)
    wts = pv / (pv.sum(-1, keepdims=True) + np.float32(EPS))

    # aux losses (all fp32)
    counts = np.bincount(idx.ravel(), minlength=E).astype(np.float32)
    tokens_per_expert = counts / np.float32(T * TOPK + EPS)
    avg_probs = probs.mean(0)
    load_balance = np.float32(E) * (tokens_per_expert * avg_probs).sum()
    lse = (m[:, 0] + np.log(ex.sum(-1)))
    z_loss = np.mean(lse ** 2) * np.float32(0.001)
    entropy = -(probs * np.log(probs + np.float32(EPS))).sum(-1).mean()
    entropy_loss = (np.log(np.float32(E)) - entropy) * np.float32(0.01)
    usage = (tokens_per_expert > 0.01).astype(np.float32).mean()
    util_loss = (np.float32(1.0) - usage) * np.float32(0.1)
    aux = np.float32(load_balance + z_loss + entropy_loss + util_loss)
    return idx, wts, aux


def _prepare(hidden_states, gate_w, expert_gate, expert_up, expert_down,
             shared_gate_w, shared_up_w, shared_down_w, shared_gate_scalar):
    """Host routing + dispatch. Returns (in_maps, meta) where meta carries
    what _combine needs."""
    x = np.ascontiguousarray(np.asarray(hidden_states, np.float32)
                             .reshape(T, H))
    gate_w = np.asarray(gate_w, np.float32)

    idx, wts, aux = _route(x, gate_w)

    # ---- dispatch: group (token, k) pairs by expert ----
    flat_e = idx.ravel()
    flat_w = wts.ravel().astype(np.float32)
    flat_t = np.repeat(np.arange(T, dtype=np.int64), TOPK)
    order = np.argsort(flat_e, kind="stable")
    counts = np.bincount(flat_e, minlength=E)
    starts = np.zeros(E + 1, np.int64)
    np.cumsum(counts, out=starts[1:])
    C = int(max(P, -(-counts.max() // P) * P))

    slot = np.empty(TOPK * T, np.int64)
    for e in range(E):
        slot[order[starts[e]:starts[e + 1]]] = np.arange(counts[e])

    x16 = x.astype(np.float16)
    sig = 1.0 / (1.0 + np.exp(-np.float32(shared_gate_scalar[0])))

    def w16T(a):  # [r, c] fp32 -> [c, r] fp16 contiguous
        return np.asarray(a, np.float32).T.astype(np.float16)

    sg16 = _pack_w(w16T(shared_gate_w))
    su16 = _pack_w(w16T(shared_up_w))
    sd16 = _pack_w((np.asarray(shared_down_w, np.float32) * sig).T
                   .astype(np.float16))

    in_maps = []
    for e in range(E):
        sel = order[starts[e]:starts[e + 1]]
        tok = flat_t[sel]
        xpad = np.zeros((C, H), np.float16)
        xpad[:counts[e]] = x16[tok]
        scv = np.zeros(C, np.float32)
        scv[:counts[e]] = flat_w[sel]
        in_maps.append({
            "xe": _pack_x(np.ascontiguousarray(xpad.T), C),
            "xs": _pack_x(np.ascontiguousarray(x16[e * SH:(e + 1) * SH].T),
                          SH),
            "sc": scv,
            "wg": _pack_w(w16T(expert_gate[e])),
            "wu": _pack_w(w16T(expert_up[e])),
            "wd": _pack_w(w16T(expert_down[e])),
            "sg": sg16, "su": su16, "sd": sd16,
        })

    contrib = (flat_e.astype(np.int64) * C + slot).reshape(T, TOPK)
    meta = {"C": C, "contrib": contrib, "aux": aux}
    return in_maps, meta


def _combine(results, meta):
    """Un-permute routed outputs (two row-gathers) + add shared output."""
    C, contrib, aux = meta["C"], meta["contrib"], meta["aux"]
    ye_all = np.concatenate([_unpack_y(results[e]["ye"], C)
                             for e in range(E)], axis=0)
    final = ye_all[contrib[:, 0]].astype(np.float32)
    final += ye_all[contrib[:, 1]].astype(np.float32)
    final += np.concatenate([_unpack_y(results[c]["ys"], SH)
                             for c in range(NCORES)], axis=0) \
        .astype(np.float32)
    final = final.reshape(B, S, H).astype(np.float32)
    return final, np.float32(aux)


def kernel(hidden_states, gate_w, expert_gate, expert_up, expert_down,
           shared_gate_w, shared_up_w, shared_down_w, shared_gate_scalar):
    in_maps, meta = _prepare(hidden_states, gate_w, expert_gate, expert_up,
                             expert_down, shared_gate_w, shared_up_w,
                             shared_down_w, shared_gate_scalar)
    C = meta["C"]
    nc = _BUILD_CACHE.get(C)
    if nc is None:
        nc = _build_nc(C)
        _BUILD_CACHE[C] = nc
    results = _execute(nc, in_maps)
    return _combine(results, meta)
